# revision 23
# baseline (speedup 1.0000x reference)
"""Trainium2 Bass kernel for nn_AttnBlock: dynamic-filter correlation.

Math (per sample b):
  p1[l, :]  = 11x11x64 patch of im1 at position l (l over 30x30)
  scores[p, l] = <im2 patch at p, p1[l] / max(||p1[l]||, 1e-4)>
  out[p] = max_l scores[p, l]

Decomposition used on device (per core = one (sample, p-half) pair):
  scores_un[p, l] = sum_{dy,dx} sum_c im2[c, p+(dy,dx)] * im1[c, l+(dy,dx)]
computed as 121 shift-matmuls (contraction over channels) accumulated in
PSUM, two shifts packed per matmul (K=128, float32r full-rate streaming).
Each image is loaded twice: partitions 0..63 hold the raw image and
partitions 64..127 hold it shifted by one element (flat +1 for the dx
pairs, flat +40 i.e. one row for the dx=10/dy pairs) via contiguous
DMAs, which bakes the pair shift into the data.  The moving operand
reads strided 2-D views of these tiles directly; the stationary operand
(walrus requires a single free dim) uses six width-30 dx-compacted
copies of im2 built with one partition-aligned DVE copy each.  Norms:
separable 11x11 box sum of im1^2 (shift-add log tree on DVE), then fp16
hi+lo ones-matmuls for the channel sum and the rank-1 partition
broadcast of 1/norm (fused fp32 matmuls silently return zeros at
M=1/K=1 on TRN2).  The two norm matmul groups are interleaved between
score chunks so the PE never idles on the DVE sqrt/reciprocal chain.
Scale + max-over-l run on DVE per PSUM tile.  GpSimd is unused: its SBUF
port is shared with DVE and concurrent use thrashes both.

Sharding: 8 cores = 4 samples x 2 halves of the output-row dim (pure
data parallel, no cross-core communication).
"""

import sys

import numpy as np

if "/opt/trn_rl_repo" not in sys.path:
    sys.path.insert(0, "/opt/trn_rl_repo")

B = 4
C = 64
H = W = 40
KER = 11
HP = WP = H - KER + 1  # 30
HALF = HP // 2  # 15 output rows per core
N_CORES = 2 * B
IM2_ROWS = HALF + KER - 1  # 25 input rows needed per half

_PROGRAM = None


def _build_program():
    import concourse.bass as bass
    import concourse.tile as tile
    from concourse import bacc

    mybir = bass.mybir
    dt = mybir.dt
    f32 = dt.float32
    f32r = dt.float32r
    f16 = dt.float16
    from contextlib import ExitStack

    nc = bacc.Bacc(
        "TRN2",
        target_bir_lowering=False,
        debug=False,
        enable_asserts=False,
        num_devices=N_CORES,
    )
    im1_d = nc.dram_tensor("im1", [C, H, W], f32, kind="ExternalInput").ap()
    im2_d = nc.dram_tensor("im2s", [C, IM2_ROWS, W], f32, kind="ExternalInput").ap()
    out_d = nc.dram_tensor("out", [128, 4], f32, kind="ExternalOutput").ap()

    MULT = mybir.AluOpType.mult
    MAX = mybir.AluOpType.max
    SQUARE = mybir.ActivationFunctionType.Square
    SQRT = mybir.ActivationFunctionType.Sqrt

    im1_flat = im1_d.rearrange("c y x -> c (y x)").bitcast(f32r)
    im2_flat = im2_d.rearrange("c y x -> c (y x)").bitcast(f32r)
    N1 = H * W
    N2 = IM2_ROWS * W

    with tile.TileContext(nc) as tc, ExitStack() as ctx:
        consts = ctx.enter_context(tc.tile_pool(name="consts", bufs=1))
        imgs = ctx.enter_context(tc.tile_pool(name="imgs", bufs=1))
        nrm = ctx.enter_context(tc.tile_pool(name="nrm", bufs=1))
        scr = ctx.enter_context(tc.tile_pool(name="scr", bufs=2))
        reds = ctx.enter_context(tc.tile_pool(name="reds", bufs=6))
        psum = ctx.enter_context(tc.tile_pool(name="psum", bufs=8, space="PSUM"))

        # Dual-shift image tiles (all contiguous DMAs).  Upper halves are
        # flat-shifted; the wrap columns/rows are never addressed by the
        # operand APs below.  The x-shift tiles gate the first matmuls, so
        # their four DMAs are spread over the sync and scalar queues; the
        # y-shift tiles are only needed ~25us in and ride the gpsimd queue.
        im2x = imgs.tile([128, IM2_ROWS, W], f32r)  # upper: flat +1 (x+1)
        nc.sync.dma_start(im2x[0:C], im2_flat)
        nc.gpsimd.dma_start(
            im2x[C : 2 * C].rearrange("p y x -> p (y x)")[:, 0 : N2 - 1],
            im2_flat[:, 1:N2],
        )
        im1x = imgs.tile([128, H, W], f32r)  # upper: flat +1 (x+1)
        nc.scalar.dma_start(im1x[0:C], im1_flat)
        im1x_up = im1x[C : 2 * C].rearrange("p y x -> p (y x)")
        nc.sync.dma_start(im1x_up[0 : C // 2, 0 : N1 - 1], im1_flat[0 : C // 2, 1:N1])
        nc.scalar.dma_start(
            im1x_up[C // 2 : C, 0 : N1 - 1], im1_flat[C // 2 : C, 1:N1]
        )
        im1y = imgs.tile([128, H, W], f32r)  # upper: flat +40 (y+1)
        im2y = imgs.tile([128, IM2_ROWS, W], f32r)  # upper: flat +40 (y+1)
        with tc.tile_wait_until(0.02):  # ~20us: keep DMA engines free for x
            nc.gpsimd.dma_start(im1y[0:C], im1_flat)
            nc.gpsimd.dma_start(
                im1y[C : 2 * C].rearrange("p y x -> p (y x)")[:, 0 : N1 - W],
                im1_flat[:, W:N1],
            )
            nc.gpsimd.dma_start(im2y[0:C], im2_flat)
            nc.gpsimd.dma_start(
                im2y[C : 2 * C].rearrange("p y x -> p (y x)")[:, 0 : N2 - W],
                im2_flat[:, W:N2],
            )

        ones_k = consts.tile([C, 1], f16)
        nc.vector.memset(ones_k[:], 1.0)
        ones_m = consts.tile([1, 128], f16)
        nc.vector.memset(ones_m[:], 1.0)

        # Warm the PE (HAM K=8/8 after ~3.4us of activity) with dummy bf16
        # matmuls while the image DMAs stream, so the real matmuls start at
        # 2.4 GHz and the HAM never sees an idle window before them.
        bf16 = dt.bfloat16
        warm_in = consts.tile([64, 64], bf16)
        with tc.high_priority():
            nc.vector.memset(warm_in[:], 0.0)
            warm_ps = psum.tile([64, 64], f32, tag="ps", name="warm_ps")
            for _ in range(220):
                nc.tensor.matmul(
                    warm_ps[:], warm_in[:], warm_in[:], start=True, stop=True
                )

        # Width-30 compacted operand tiles: the stationary side must be a
        # single-free-dim AP, and a contiguous moving side streams ~6%
        # faster than strided reads.  One partition-aligned copy per tile
        # (pair shift already baked into the source's upper half); c2 on
        # GpSimd, c1 on DVE, all at high priority so they precede the norm
        # tree in the engine streams.  The y-shifted (dx=10) sources hold
        # one row less in the upper half, so those copies are split.
        dx_bases = [0, 2, 4, 6, 8, 10]
        im1c = []
        im2c = []
        with tc.high_priority():
            for bi, dx in enumerate(dx_bases):
                c2 = imgs.tile([128, IM2_ROWS, WP], f32r, name=f"im2c_{bi}")
                if dx < 10:
                    nc.vector.tensor_copy(c2[:], im2x[:, :, dx : dx + WP])
                else:
                    nc.vector.tensor_copy(c2[0:C], im2y[0:C, :, dx : dx + WP])
                    nc.vector.tensor_copy(
                        c2[C : 2 * C, 0 : IM2_ROWS - 1, :],
                        im2y[C : 2 * C, 0 : IM2_ROWS - 1, dx : dx + WP],
                    )
                im2c.append(c2)
                c1 = imgs.tile([128, H, WP], f32r, name=f"im1c_{bi}")
                if dx < 10:
                    nc.vector.tensor_copy(c1[:], im1x[:, :, dx : dx + WP])
                else:
                    nc.vector.tensor_copy(c1[0:C], im1y[0:C, :, dx : dx + WP])
                    nc.vector.tensor_copy(
                        c1[C : 2 * C, 0 : H - 1, :],
                        im1y[C : 2 * C, 0 : H - 1, dx : dx + WP],
                    )
                im1c.append(c1)

        def rhs_ap(bi, dx, kp, y0):
            return im1c[bi][0:kp, y0 : y0 + HALF, :]

        # ---- norm DVE chain: separable 11x11 box sum of im1^2 over (y, x).
        # Shift-add log tree: widths 1->2->4->8->11.
        sq = nrm.tile([C, H, W], f32)
        nc.scalar.activation(sq[:], im1x[0:C].bitcast(f32), SQUARE)

        t2 = nrm.tile([C, H, W - 1], f32)
        nc.vector.tensor_add(t2[:], sq[:, :, 0 : W - 1], sq[:, :, 1:W])
        t4 = nrm.tile([C, H, W - 3], f32)
        nc.vector.tensor_add(t4[:], t2[:, :, 0 : W - 3], t2[:, :, 2 : W - 1])
        t8 = nrm.tile([C, H, W - 7], f32)
        nc.vector.tensor_add(t8[:], t4[:, :, 0 : W - 7], t4[:, :, 4 : W - 3])
        rp_a = nrm.tile([C, H, WP], f32)
        nc.vector.tensor_add(rp_a[:], t8[:, :, 0:WP], t2[:, :, 8 : 8 + WP])
        rp = nrm.tile([C, H, WP], f32)
        nc.vector.tensor_add(rp[:], rp_a[:], sq[:, :, 10 : 10 + WP])

        u2 = nrm.tile([C, H - 1, WP], f32)
        nc.vector.tensor_add(u2[:], rp[:, 0 : H - 1], rp[:, 1:H])
        u4 = nrm.tile([C, H - 3, WP], f32)
        nc.vector.tensor_add(u4[:], u2[:, 0 : H - 3], u2[:, 2 : H - 1])
        u8 = nrm.tile([C, H - 7, WP], f32)
        nc.vector.tensor_add(u8[:], u4[:, 0 : H - 7], u4[:, 4 : H - 3])
        nc_a = nrm.tile([C, HP, WP], f32)
        nc.vector.tensor_add(nc_a[:], u8[:, 0:HP], u2[:, 8 : 8 + HP])
        normc = nrm.tile([C, HP, WP], f32)
        nc.vector.tensor_add(normc[:], nc_a[:], rp[:, 10 : 10 + HP])

        # fp32 -> fp16 hi + lo residual pair (for exact-ish fp16 matmuls).
        def split_f16(src_ap, pool, parts, n, stem):
            hi = pool.tile([parts, n], f16, name=f"{stem}_hi")
            nc.vector.tensor_copy(hi[:], src_ap)
            back = pool.tile([parts, n], f32, name=f"{stem}_back")
            nc.vector.tensor_copy(back[:], hi[:])
            res32 = pool.tile([parts, n], f32, name=f"{stem}_r32")
            nc.vector.tensor_sub(res32[:], src_ap, back[:])
            lo = pool.tile([parts, n], f16, name=f"{stem}_lo")
            nc.vector.tensor_copy(lo[:], res32[:])
            return hi, lo

        NL = HALF * WP  # 450: l columns per l-chunk
        ncv = normc[:].rearrange("p y x -> p (y x)")
        normc_hi, normc_lo = split_f16(ncv, nrm, C, 2 * NL, "normc")

        # ---- main correlation matmuls.  121 shifts = 60 packed pairs + 1
        # K=64 single (dy=10, dx=10).
        row_chunks = [(0, 4), (4, 4), (8, 4), (12, 3)]

        def emit_chunk_mms(r0, nr):
            M = nr * WP
            ps = [
                psum.tile([128, NL], f32, tag="ps", name=f"ps_{r0}_{j}")
                for j in range(2)
            ]
            for j in range(2):
                first = True
                for bi, dx in enumerate(dx_bases):
                    dys = range(KER) if dx < 10 else range(0, KER, 2)
                    for dy in dys:
                        kp = C if (dx == 10 and dy == 10) else 2 * C
                        lhsT = im2c[bi][0:kp, r0 + dy : r0 + dy + nr, :]
                        last = dx == 10 and dy == 10
                        rhs = rhs_ap(bi, dx, kp, HALF * j + dy)
                        nc.tensor.matmul(ps[j][0:M], lhsT, rhs, start=first, stop=last)
                        first = False
            return ps

        red_all = reds.tile([128, 4], f32, name="red_all")
        nc.vector.memset(red_all[:], 0.0)

        def emit_epilogue(ci, r0, nr, ps):
            M = nr * WP
            sc0 = scr.tile([128, NL], f32, tag="sc", name=f"sc0_{r0}")
            sc1 = scr.tile([128, NL], f32, tag="sc", name=f"sc1_{r0}")
            red0 = reds.tile([128, 1], f32, tag="red", name=f"red0_{r0}")
            red1 = reds.tile([128, 1], f32, tag="red", name=f"red1_{r0}")
            nc.vector.tensor_tensor(
                out=sc0[0:M], in0=ps[0][0:M], in1=inv_bc[0:M, 0:NL], op=MULT
            )
            nc.vector.tensor_reduce(
                out=red0[0:M], in_=sc0[0:M], axis=mybir.AxisListType.X, op=MAX
            )
            nc.vector.tensor_tensor(
                out=sc1[0:M], in0=ps[1][0:M], in1=inv_bc[0:M, NL : 2 * NL], op=MULT
            )
            nc.vector.tensor_reduce(
                out=red1[0:M], in_=sc1[0:M], axis=mybir.AxisListType.X, op=MAX
            )
            nc.vector.tensor_tensor(
                out=red_all[0:M, ci : ci + 1], in0=red0[0:M], in1=red1[0:M], op=MAX
            )

        chunk_ps = {}
        chunk_ps[0] = emit_chunk_mms(*row_chunks[0])
        chunk_ps[1] = emit_chunk_mms(*row_chunks[1])

        # norm matmul group 1: fp16 hi+lo channel sum -> sqrt.  Placed two
        # score chunks in so the DVE tree is long done when the PE gets
        # here; the chain (sqrt -> clamp -> reciprocal -> split) then runs
        # during chunk 2, and the broadcast group lands after it.
        inv_s = nrm.tile([1, 2 * NL], f32)
        for j in range(2):
            nm = psum.tile([1, NL], f32, tag="ps", name=f"nm_{j}")
            sl = slice(NL * j, NL * (j + 1))
            nc.tensor.matmul(nm[:], ones_k[:], normc_hi[:, sl], start=True, stop=False)
            nc.tensor.matmul(nm[:], ones_k[:], normc_lo[:, sl], start=False, stop=True)
            nc.scalar.activation(inv_s[:, sl], nm[:], SQRT)

        chunk_ps[2] = emit_chunk_mms(*row_chunks[2])

        nc.vector.tensor_scalar_max(inv_s[:], inv_s[:], 1e-4)
        nc.vector.reciprocal(inv_s[:], inv_s[:])
        inv_hi, inv_lo = split_f16(inv_s[:], nrm, 1, 2 * NL, "inv")

        inv_bc = nrm.tile([128, 2 * NL], f32)
        for j in range(2):
            ip = psum.tile([128, NL], f32, tag="ps", name=f"ip_{j}")
            sl = slice(NL * j, NL * (j + 1))
            nc.tensor.matmul(ip[:], ones_m[:], inv_hi[:, sl], start=True, stop=False)
            nc.tensor.matmul(ip[:], ones_m[:], inv_lo[:, sl], start=False, stop=True)
            nc.vector.tensor_copy(inv_bc[:, sl], ip[:])

        emit_epilogue(0, *row_chunks[0], chunk_ps[0])
        chunk_ps[3] = emit_chunk_mms(*row_chunks[3])
        emit_epilogue(1, *row_chunks[1], chunk_ps[1])
        emit_epilogue(2, *row_chunks[2], chunk_ps[2])
        emit_epilogue(3, *row_chunks[3], chunk_ps[3])
        nc.gpsimd.dma_start(out_d, red_all[:])

    nc.compile()
    return nc


def _get_program():
    global _PROGRAM
    if _PROGRAM is None:
        _PROGRAM = _build_program()
    return _PROGRAM


def make_in_maps(im1: np.ndarray, im2: np.ndarray):
    in_maps = []
    for b in range(B):
        for h in range(2):
            in_maps.append(
                {
                    "im1": np.ascontiguousarray(im1[b], dtype=np.float32),
                    "im2s": np.ascontiguousarray(
                        im2[b][:, HALF * h : HALF * h + IM2_ROWS, :], dtype=np.float32
                    ),
                }
            )
    return in_maps


ROW_CHUNKS = [(0, 4), (4, 4), (8, 4), (12, 3)]


def _half_from_cols(cols):
    half = np.empty((HALF * WP,), dtype=np.float32)
    for ci, (r0, nr) in enumerate(ROW_CHUNKS):
        half[WP * r0 : WP * r0 + nr * WP] = cols[0 : nr * WP, ci]
    return half.reshape(HALF, WP)


def assemble(results):
    out = np.empty((B, 1, HP, WP), dtype=np.float32)
    for b in range(B):
        top = _half_from_cols(results[2 * b]["out"])
        bot = _half_from_cols(results[2 * b + 1]["out"])
        out[b, 0] = np.concatenate([top, bot], axis=0)
    return out


def run(im1: np.ndarray, im2: np.ndarray, trace: bool = False):
    from concourse import bass_utils

    nc = _get_program()
    res = bass_utils.run_bass_kernel_spmd(
        nc, make_in_maps(im1, im2), core_ids=list(range(N_CORES)), trace=trace
    )
    return assemble(res.results), res


def kernel(im1: np.ndarray, im2: np.ndarray) -> np.ndarray:
    out, _ = run(np.asarray(im1), np.asarray(im2))
    return out
